# revision 4
# baseline (speedup 1.0000x reference)
"""Trainium2 Bass kernel for nn_BlockRF (BatchNorm -> LocallyConnected2D 3x3 valid -> ReLU).

Shapes (hardcoded per the problem spec):
  x:      [B=32, H=64, W=64, C=32]  f32
  gamma/beta/moving_mean/moving_var: [C=32] f32
  weight: [OH*OW=3844, KH*KW*C=288, F=32] f32
  out:    [B=32, OH=62, OW=62, F=32] f32

Strategy: shard over output rows (OH). OH=62 padded to 64 -> 8 rows/core on 8
cores. Each core streams only its slice of the (dominant) weight tensor.

Per core:
  - The 8 output rows' input windows (rows r0..r0+2 each, host-duplicated)
    live in one SBUF tile X[(i,c)=96 partitions, (oh,w,b)], BN-normalized
    once in fp16.
  - Weights stream in 4 blocks of 2 output rows: W[(i,c)=96, (ohl,w,g,f)]
    fp16, where slot (w,g) holds the 3x32-channel chunk j=2-g of position
    ow=w-2+g (so the matmul for (ow, j) reads slot (w=ow+j, g=2-j)).
  - For each position ow: 3 accumulating matmuls (K=96=3 taps x 32ch,
    M=B=32, N=F=32) into a PSUM slice; lhsT = X w-slice (stationary),
    rhs = weight chunk (moving). fp16 operands, fp32 PSUM accumulation.
  - PSUM tiles hold 32 positions (2 banks); ReLU evacuation alternates
    between VectorE (tensor_scalar_max) and ScalarE (activation Relu).
  - One output DMA per 2-oh block.

Host side only pads/transposes/casts (layout prep + sharding) - all model
arithmetic (BN, conv, ReLU) runs on device.
"""

import numpy as np

B, H, W, C, F = 32, 64, 64, 32, 32
KH = KW = 3
OH = OW = 62
OHP = 64  # padded OH
RPC = OHP // 8  # output rows per core = 8
OHB = 2  # output rows per weight block
NBLK = RPC // OHB  # 4 blocks
EPS = 1e-3
NPART = KH * C  # 96 partitions = (i, c)
XFREE = W * B  # 2048 per oh
WSLOT = (OW + 2) * 3 * F  # 6144 per oh
YFREE = OW * F  # 1984 per oh
PSUM_POS = 32  # positions per PSUM tile (32*32*4B = 4KB = two banks)

_CACHE = {}


def _build_program():
    import concourse.mybir as mybir
    import concourse.tile as tile
    from concourse import bacc
    from contextlib import ExitStack

    f16 = mybir.dt.float16
    f32 = mybir.dt.float32

    nc = bacc.Bacc("TRN2", target_bir_lowering=False, debug=False, num_devices=8)

    xin = nc.dram_tensor("xin", [NPART, RPC * XFREE], f16, kind="ExternalInput").ap()
    win = nc.dram_tensor(
        "win", [NBLK, NPART, OHB * WSLOT], f16, kind="ExternalInput"
    ).ap()
    pin = nc.dram_tensor("pin", [NPART, 4], f32, kind="ExternalInput").ap()
    yout = nc.dram_tensor(
        "yout", [NBLK, B, OHB * YFREE], f32, kind="ExternalOutput"
    ).ap()

    with ExitStack() as ctx:
        tc = ctx.enter_context(tile.TileContext(nc))
        singles = ctx.enter_context(tc.tile_pool(name="singles", bufs=1))
        wpool = ctx.enter_context(tc.tile_pool(name="wpool", bufs=3))
        opool = ctx.enter_context(tc.tile_pool(name="opool", bufs=2))
        pspool = ctx.enter_context(
            tc.tile_pool(name="pspool", bufs=4, space="PSUM")
        )

        # ---- BN affine params: A = gamma/sqrt(var+eps), Bb = beta - mean*A
        par = singles.tile([NPART, 4], f32)
        nc.sync.dma_start(out=par, in_=pin)
        tmp = singles.tile([NPART, 1], f32)
        A = singles.tile([NPART, 1], f32)
        Bb = singles.tile([NPART, 1], f32)
        nc.vector.tensor_scalar_add(tmp, par[:, 3:4], EPS)  # var + eps
        nc.scalar.sqrt(tmp, tmp)
        nc.vector.reciprocal(A, tmp)  # 1/sqrt(var+eps)
        nc.vector.tensor_mul(A, A, par[:, 0:1])  # * gamma
        nc.vector.tensor_mul(tmp, A, par[:, 2:3])  # mean * A
        nc.vector.tensor_sub(Bb, par[:, 1:2], tmp)  # beta - mean*A

        # ---- x: one DMA + one BN over all 8 rows' windows
        xt = singles.tile([NPART, RPC * XFREE], f16)
        nc.sync.dma_start(out=xt, in_=xin)
        xn = singles.tile([NPART, RPC * XFREE], f16)
        nc.vector.tensor_scalar(
            xn, xt, A, Bb,
            op0=mybir.AluOpType.mult, op1=mybir.AluOpType.add,
        )

        for blk in range(NBLK):
            wt = wpool.tile([NPART, OHB * WSLOT], f16)
            nc.sync.dma_start(out=wt, in_=win[blk])
            rowbuf = opool.tile([B, OHB * YFREE], f32)
            for ohl in range(OHB):
                oh = blk * OHB + ohl
                for grp in range((OW + PSUM_POS - 1) // PSUM_POS):
                    npos = min(PSUM_POS, OW - grp * PSUM_POS)
                    ps = pspool.tile([B, PSUM_POS * F], mybir.dt.float32)
                    for s in range(npos):
                        ow = grp * PSUM_POS + s
                        for j in range(3):
                            w = ow + j
                            g = 2 - j
                            nc.tensor.matmul(
                                ps[:, s * F:(s + 1) * F],
                                xn[:, (oh * W + w) * B:(oh * W + w + 1) * B],
                                wt[:, (ohl * 3 * W + w * 3 + g) * F
                                   :(ohl * 3 * W + w * 3 + g + 1) * F],
                                start=(j == 0),
                                stop=(j == 2),
                            )
                    dst = rowbuf[:, ohl * YFREE + grp * PSUM_POS * F
                                 : ohl * YFREE + grp * PSUM_POS * F + npos * F]
                    if grp % 2 == 0:
                        nc.vector.tensor_scalar_max(dst, ps[:, : npos * F], 0.0)
                    else:
                        nc.scalar.activation(
                            dst, ps[:, : npos * F],
                            mybir.ActivationFunctionType.Relu,
                        )
            nc.sync.dma_start(out=yout[blk], in_=rowbuf)

    nc.compile()
    return nc


def _get_program():
    if "nc" not in _CACHE:
        _CACHE["nc"] = _build_program()
    return _CACHE["nc"]


def _prep_inputs(x, gamma, beta, moving_mean, moving_var, weight):
    """Host-side shard/layout/cast prep. Returns per-core in_maps."""
    x = np.asarray(x, dtype=np.float32)
    weight = np.asarray(weight, dtype=np.float32)

    # x: [B,H,W,C] -> pad H to 66 -> transpose to (h, c, w, b), fp16
    xpad = np.zeros((B, H + 2, W, C), np.float32)
    xpad[:, :H] = x
    xt_all = np.ascontiguousarray(xpad.transpose(1, 3, 2, 0)).astype(np.float16)

    # weight: [3844, 288, 32] -> (oh, ow, i, j, c, f) -> (oh, i, c, ow, j, f)
    w6 = weight.reshape(OH, OW, KH, KW, C, F)
    wtr = np.ascontiguousarray(w6.transpose(0, 2, 4, 1, 3, 5)).astype(np.float16)
    # wg[oh, i, c, w, g, f]: slot (w, g) = position ow=w-2+g, tap-col j=2-g
    wg = np.zeros((OHP, KH, C, OW + 2, 3, F), np.float16)
    for g in range(3):
        j = 2 - g
        wg[:OH, :, :, j:j + OW, g, :] = wtr[:, :, :, :, j, :]

    p96 = np.tile(
        np.stack([gamma, beta, moving_mean, moving_var], axis=1).astype(np.float32),
        (KH, 1),
    )  # [96, 4]

    in_maps = []
    for k in range(8):
        R = k * RPC
        xc = np.concatenate(
            [xt_all[R + oh: R + oh + 3].reshape(NPART, XFREE) for oh in range(RPC)],
            axis=1,
        )  # [96, 8*2048]
        # [oh, i, c, w, g, f] -> blocks [blk, (i,c), (ohl, w, g, f)]
        wk = wg[R: R + RPC].reshape(NBLK, OHB, KH * C, WSLOT)
        wc = np.ascontiguousarray(wk.transpose(0, 2, 1, 3)).reshape(
            NBLK, NPART, OHB * WSLOT
        )
        in_maps.append({"xin": xc, "win": wc, "pin": p96})
    return in_maps


def _assemble_output(results):
    """results: list (per core) of {"yout": [NBLK, B, OHB*YFREE]} -> [B,OH,OW,F]."""
    rows = []
    for r in results:
        a = r["yout"].reshape(NBLK, B, OHB, OW, F).transpose(0, 2, 1, 3, 4)
        rows.append(a.reshape(RPC, B, OW, F))
    yall = np.concatenate(rows, axis=0)  # [64, B, OW, F]
    y = yall.transpose(1, 0, 2, 3)[:, :OH]
    return np.ascontiguousarray(y)


def run(inputs, trace=False, trace_cores=None):
    """Build/compile/run on 8 cores. Returns (y, BassKernelResults)."""
    from concourse.bass_utils import run_bass_kernel_spmd

    nc = _get_program()
    in_maps = _prep_inputs(**inputs)
    res = run_bass_kernel_spmd(
        nc,
        in_maps,
        core_ids=list(range(8)),
        trace=trace,
        **({"trace_cores": trace_cores} if trace_cores is not None else {}),
    )
    return _assemble_output(res.results), res


def kernel(x, gamma, beta, moving_mean, moving_var, weight):
    y, _ = run(
        dict(x=x, gamma=gamma, beta=beta, moving_mean=moving_mean,
             moving_var=moving_var, weight=weight)
    )
    return y


# revision 5
# speedup vs baseline: 1.1533x; 1.1533x over previous
"""Trainium2 Bass kernel for nn_BlockRF (BatchNorm -> LocallyConnected2D 3x3 valid -> ReLU).

Shapes (hardcoded per the problem spec):
  x:      [B=32, H=64, W=64, C=32]  f32
  gamma/beta/moving_mean/moving_var: [C=32] f32
  weight: [OH*OW=3844, KH*KW*C=288, F=32] f32
  out:    [B=32, OH=62, OW=62, F=32] f32

Strategy: shard over output rows (OH). OH=62 padded to 64 -> 8 rows/core on 8
cores. Each core streams only its slice of the (dominant) weight tensor.

Per core, per output row oh (pipelined via tile pools):
  - x rows r0..r0+2 live in an SBUF tile X[(i,c)=96, (w,b)=2048], BN applied
    in fp16 (VectorE).
  - weights stream per-oh: W[(i,c)=96, (w,g,f)=6144] fp16, where slot (w,g)
    holds the 3x32-channel chunk j=2-g of position ow=w-2+g (the matmul for
    (ow, j) reads slot (w=ow+j, g=2-j)).
  - For each position ow: 3 accumulating matmuls (K=96=3 taps x 32ch,
    M=B=32, N=F=32) into a PSUM slice; lhsT = X w-slice (stationary),
    rhs = weight chunk (moving). fp16 operands, fp32 PSUM accumulation.
  - PSUM tiles hold 32 positions (2 banks); ReLU evacuation (fp16 out)
    alternates between VectorE and ScalarE.
  - Per-oh fp16 output DMA on the scalar HWDGE ring (keeps the sync ring
    exclusively for input streaming); host upcasts to fp32.

Host side only pads/transposes/casts (layout prep + sharding) - all model
arithmetic (BN, conv, ReLU) runs on device.
"""

import numpy as np

B, H, W, C, F = 32, 64, 64, 32, 32
KH = KW = 3
OH = OW = 62
OHP = 64  # padded OH
RPC = OHP // 8  # output rows per core = 8
EPS = 1e-3
NPART = KH * C  # 96 partitions = (i, c)
XFREE = W * B  # 2048
WSLOT = (OW + 2) * 3 * F  # 6144
YFREE = OW * F  # 1984
PSUM_POS = 32  # positions per PSUM tile (32*32*4B = 4KB = two banks)

_CACHE = {}


def _build_program():
    import concourse.mybir as mybir
    import concourse.tile as tile
    from concourse import bacc
    from contextlib import ExitStack

    f16 = mybir.dt.float16
    f32 = mybir.dt.float32

    nc = bacc.Bacc("TRN2", target_bir_lowering=False, debug=False, num_devices=8)

    xin = nc.dram_tensor("xin", [RPC, NPART, XFREE], f16, kind="ExternalInput").ap()
    win = nc.dram_tensor("win", [RPC, NPART, WSLOT], f16, kind="ExternalInput").ap()
    pin = nc.dram_tensor("pin", [NPART, 4], f32, kind="ExternalInput").ap()
    yout = nc.dram_tensor("yout", [RPC, B, YFREE], f16, kind="ExternalOutput").ap()

    with ExitStack() as ctx:
        tc = ctx.enter_context(tile.TileContext(nc))
        singles = ctx.enter_context(tc.tile_pool(name="singles", bufs=1))
        xpool = ctx.enter_context(tc.tile_pool(name="xpool", bufs=3))
        xnpool = ctx.enter_context(tc.tile_pool(name="xnpool", bufs=3))
        wpool = ctx.enter_context(tc.tile_pool(name="wpool", bufs=4))
        opool = ctx.enter_context(tc.tile_pool(name="opool", bufs=3))
        pspool = ctx.enter_context(
            tc.tile_pool(name="pspool", bufs=4, space="PSUM")
        )

        # ---- BN affine params: A = gamma/sqrt(var+eps), Bb = beta - mean*A
        par = singles.tile([NPART, 4], f32)
        nc.sync.dma_start(out=par, in_=pin)
        tmp = singles.tile([NPART, 1], f32)
        A = singles.tile([NPART, 1], f32)
        Bb = singles.tile([NPART, 1], f32)
        nc.vector.tensor_scalar_add(tmp, par[:, 3:4], EPS)  # var + eps
        nc.scalar.sqrt(tmp, tmp)
        nc.vector.reciprocal(A, tmp)  # 1/sqrt(var+eps)
        nc.vector.tensor_mul(A, A, par[:, 0:1])  # * gamma
        nc.vector.tensor_mul(tmp, A, par[:, 2:3])  # mean * A
        nc.vector.tensor_sub(Bb, par[:, 1:2], tmp)  # beta - mean*A

        for oh in range(RPC):
            wt = wpool.tile([NPART, WSLOT], f16)
            nc.sync.dma_start(out=wt, in_=win[oh])
            xt = xpool.tile([NPART, XFREE], f16)
            nc.sync.dma_start(out=xt, in_=xin[oh])
            xn = xnpool.tile([NPART, XFREE], f16)
            nc.vector.tensor_scalar(
                xn, xt, A, Bb,
                op0=mybir.AluOpType.mult, op1=mybir.AluOpType.add,
            )

            rowbuf = opool.tile([B, YFREE], f16)
            for grp in range((OW + PSUM_POS - 1) // PSUM_POS):
                npos = min(PSUM_POS, OW - grp * PSUM_POS)
                ps = pspool.tile([B, PSUM_POS * F], mybir.dt.float32)
                for s in range(npos):
                    ow = grp * PSUM_POS + s
                    for j in range(3):
                        w = ow + j
                        g = 2 - j
                        nc.tensor.matmul(
                            ps[:, s * F:(s + 1) * F],
                            xn[:, w * B:(w + 1) * B],
                            wt[:, (w * 3 + g) * F:(w * 3 + g + 1) * F],
                            start=(j == 0),
                            stop=(j == 2),
                        )
                dst = rowbuf[:, grp * PSUM_POS * F
                             : grp * PSUM_POS * F + npos * F]
                if grp % 2 == 0:
                    nc.vector.tensor_scalar_max(dst, ps[:, : npos * F], 0.0)
                else:
                    nc.scalar.activation(
                        dst, ps[:, : npos * F],
                        mybir.ActivationFunctionType.Relu,
                    )
            nc.scalar.dma_start(out=yout[oh], in_=rowbuf)

    nc.compile()
    return nc


def _get_program():
    if "nc" not in _CACHE:
        _CACHE["nc"] = _build_program()
    return _CACHE["nc"]


def _prep_inputs(x, gamma, beta, moving_mean, moving_var, weight):
    """Host-side shard/layout/cast prep. Returns per-core in_maps."""
    x = np.asarray(x, dtype=np.float32)
    weight = np.asarray(weight, dtype=np.float32)

    # x: [B,H,W,C] -> pad H to 66 -> transpose to (h, c, w, b), fp16
    xpad = np.zeros((B, H + 2, W, C), np.float32)
    xpad[:, :H] = x
    xt_all = np.ascontiguousarray(xpad.transpose(1, 3, 2, 0)).astype(np.float16)

    # weight: [3844, 288, 32] -> (oh, ow, i, j, c, f) -> (oh, i, c, ow, j, f)
    w6 = weight.reshape(OH, OW, KH, KW, C, F)
    wtr = np.ascontiguousarray(w6.transpose(0, 2, 4, 1, 3, 5)).astype(np.float16)
    # wg[oh, i, c, w, g, f]: slot (w, g) = position ow=w-2+g, tap-col j=2-g
    wg = np.zeros((OHP, KH, C, OW + 2, 3, F), np.float16)
    for g in range(3):
        j = 2 - g
        wg[:OH, :, :, j:j + OW, g, :] = wtr[:, :, :, :, j, :]

    p96 = np.tile(
        np.stack([gamma, beta, moving_mean, moving_var], axis=1).astype(np.float32),
        (KH, 1),
    )  # [96, 4]

    in_maps = []
    for k in range(8):
        R = k * RPC
        xc = np.stack(
            [xt_all[R + oh: R + oh + 3].reshape(NPART, XFREE) for oh in range(RPC)]
        )  # [8, 96, 2048]
        wc = np.ascontiguousarray(wg[R: R + RPC]).reshape(RPC, NPART, WSLOT)
        in_maps.append({"xin": xc, "win": wc, "pin": p96})
    return in_maps


def _assemble_output(results):
    """results: list (per core) of {"yout": [RPC, B, YFREE] f16} -> [B,OH,OW,F] f32."""
    yall = np.concatenate([r["yout"] for r in results], axis=0)  # [64, B, YFREE]
    y = yall.astype(np.float32).reshape(OHP, B, OW, F).transpose(1, 0, 2, 3)[:, :OH]
    return np.ascontiguousarray(y)


def run(inputs, trace=False, trace_cores=None):
    """Build/compile/run on 8 cores. Returns (y, BassKernelResults)."""
    from concourse.bass_utils import run_bass_kernel_spmd

    nc = _get_program()
    in_maps = _prep_inputs(**inputs)
    res = run_bass_kernel_spmd(
        nc,
        in_maps,
        core_ids=list(range(8)),
        trace=trace,
        **({"trace_cores": trace_cores} if trace_cores is not None else {}),
    )
    return _assemble_output(res.results), res


def kernel(x, gamma, beta, moving_mean, moving_var, weight):
    y, _ = run(
        dict(x=x, gamma=gamma, beta=beta, moving_mean=moving_mean,
             moving_var=moving_var, weight=weight)
    )
    return y
